# revision 75
# baseline (speedup 1.0000x reference)
"""GNN message-passing kernel for 8 Trainium2 NeuronCores (Bass/Tile).

Sharding: each core owns 2500 nodes + all edges targeting them. Node state
is feature-major in SBUF; after each GRU update it is PE-transposed to
node-major (bf16, 128-wide rows), DMAd to DRAM and AllGathered (Shared
scratchpad) so any core can dma_gather arbitrary source rows.  Per-edge
weights are never materialized:
  msg_e = h[src_e] @ (ea_e @ nnW^T).reshape(D,D)
is computed as  Z[(k,i),e] = ea[k,e] * h[i,src_e];  msg = G^T @ Z
with G a host-prepacked rearrangement of nnW.  segment-sum over targets is
a matmul against host-built staircase blocks holding 1/deg of the target
(edges sorted by target, each 512-node tile padded to an integral number
of 128-edge chunks).
"""
import sys
sys.path.insert(0, "/opt/trn_rl_repo")
import os
import numpy as np
import ml_dtypes

import concourse.bass as bass
import concourse.bacc as bacc
import concourse.mybir as mybir
import concourse.tile as tile
from concourse.bass_utils import run_bass_kernel_spmd

F32 = mybir.dt.float32
BF16 = mybir.dt.bfloat16
I16 = mybir.dt.int16
AF = mybir.ActivationFunctionType
ALU = mybir.AluOpType

N, E, E3, D = 20000, 30000, 4000, 64
D2 = 2 * D
C = 8
NL = N // C          # nodes per core (2500)
NLP = 2560           # padded rows per core in gathered state (mult of 128)
NT = 512             # node-tile / matmul moving chunk
NTC = (NL + NT - 1) // NT
SLOPE = (1.0 / 8.0 + 1.0 / 3.0) / 2.0
EPS = 1e-5
EF = 512             # final readout edges per core (500 real)

bfd = ml_dtypes.bfloat16


# ----------------------------------------------------------------- host prep

def _wrap16(idx):
    n = len(idx)
    w = idx.reshape(n // 16, 16).T.astype(np.int16)
    return np.tile(w, (8, 1)).copy()


def _pad_id(n):
    """global node id -> padded row id in gathered state."""
    return (n // NL) * NLP + (n % NL)


def _affine_bn(g, be, m, v):
    a = g / np.sqrt(v + EPS)
    return a, be - m * a


def _balance_perm(tgt1):
    """Per-core position of each node, degree-balanced across tiles.

    Returns pos[n] in [0, NL): row of node n within its core, chosen so
    stage-1 in-edges spread evenly over the NTC 512-node tiles."""
    deg = np.bincount(tgt1, minlength=N)
    pos = np.empty(N, np.int64)
    for c in range(C):
        nodes = np.arange(c * NL, (c + 1) * NL)
        order = nodes[np.argsort(-deg[nodes], kind="stable")]
        fill = np.zeros(NTC, np.int64)
        cap = [min(NT, NL - t * NT) for t in range(NTC)]
        for i, n in enumerate(order):
            t = i % NTC
            while fill[t] >= cap[t]:
                t = (t + 1) % NTC
            pos[n] = t * NT + fill[t]
            fill[t] += 1
    return pos


HSA = 1536           # rows per rank in half-a state (tiles 0-2)
HSB = 1024           # rows per rank in half-b state (tiles 3-4, padded)


def _prep_edges(src, tgt, attr, n_attr, pos, deg):
    """Bucket edges per target tile (balanced via pos), each bucket padded
    to a 128 multiple; staircase S blocks carry 1/deg of the target."""
    owner = tgt // NL
    per_core = []
    for c in range(C):
        sel = np.where(owner == c)[0]
        tl = pos[tgt[sel]]
        sh = np.zeros(len(sel), np.int64)        # single run (no half split)
        order = np.lexsort((tl, tl // NT, sh))
        per_core.append((sel[order], tl[order], sh[order]))

    # shared static layout: per (half, tile) block counts = max over cores
    nblk = np.zeros((2, NTC), np.int64)
    for sel, tl, sh in per_core:
        for h in (0, 1):
            for t in range(NTC):
                k = int(((sh == h) & (tl // NT == t)).sum())
                nblk[h, t] = max(nblk[h, t], (k + 127) // 128)
    nblk[0] = np.maximum(nblk[0], 1)   # >=1 chunk so psum init happens
    ep = int(nblk.sum()) * 128

    # two contiguous runs (all half-a buckets, then all half-b); per tile
    # the chunk spans it owns in each half
    runs = []
    spans = [[] for _ in range(NTC)]
    off = 0
    for h in (0, 1):
        ln = int(nblk[h].sum()) * 128
        if ln:
            runs.append((off, ln, h))
        for t in range(NTC):
            n = int(nblk[h, t])
            if n:
                spans[t].append((off // 128, n))
            off += n * 128
    tcs = tuple(tuple(s) for s in spans)

    gidx = np.zeros((C, ep), np.int64)
    eaT = np.zeros((C, n_attr, ep), np.float32)
    s_blocks = np.zeros((C, ep // 128, 128, NT), bfd)

    for c, (sel, tl, sh) in enumerate(per_core):
        off = 0
        for h in (0, 1):
            for t in range(NTC):
                msk = (sh == h) & (tl // NT == t)
                idxs, tls = sel[msk], tl[msk]
                k = len(idxs)
                gidx[c, off:off + k] = _pad_id_pos(src[idxs], pos)
                eaT[c, :, off:off + k] = attr[idxs].T
                rel = tls - t * NT
                ar = np.arange(k) + off
                s_blocks[c, ar // 128, ar % 128, rel] = (
                    1.0 / deg[tgt[idxs]]).astype(bfd)
                off += int(nblk[h, t]) * 128
    return dict(ep=ep, gidx=gidx, eaT=eaT, s=s_blocks,
                runs=tuple(runs), tcs=tcs)


def _pad_id_pos(n, pos):
    """global node id -> padded row id in unsplit gathered state."""
    return (n // NL) * NLP + pos[n]


def _host_prep(inp):
    g = lambda k: np.asarray(inp[k], np.float32)
    ei = np.asarray(inp["edge_index"], np.int64)
    ei3 = np.asarray(inp["edge_index3"], np.int64)

    a, b = _affine_bn(g("nx_g"), g("nx_be"), g("nx_m"), g("nx_v"))
    Wln = (a[:, None] * g("ln_W").T).astype(np.float32)
    bln = (b @ g("ln_W").T + g("ln_b")).astype(np.float32)

    pos = _balance_perm(ei[1])
    deg1 = np.maximum(np.bincount(ei[1], minlength=N), 1).astype(np.float32)
    e1 = _prep_edges(ei[0], ei[1], g("edge_attr"), 19, pos, deg1)
    nn1 = g("nn1_W")
    G1 = np.zeros((128, 6, 64), np.float32)           # partition-first
    REP1 = np.zeros((12, 6, 128), np.float32)
    for cc in range(6):
        for half, k in enumerate((2 * cc, 2 * cc + 1)):
            G1[half * 64:(half + 1) * 64, cc, :] = nn1[:, k].reshape(64, 64)
            REP1[k, cc, half * 64:(half + 1) * 64] = 1.0

    src3 = np.concatenate([ei3[0], ei3[1]])
    tgt3 = np.concatenate([ei3[1], ei3[0]])
    attr3 = np.concatenate([g("edge_attr3"), g("edge_attr3")], axis=0)
    deg3 = np.maximum(np.bincount(tgt3, minlength=N), 1).astype(np.float32)
    e2 = _prep_edges(src3, tgt3, attr3, 8, pos, deg3)
    nn2 = g("nn2_W")
    G2 = np.zeros((128, 8, 128), np.float32)
    REP2 = np.zeros((8, 8, 128), np.float32)
    for k in range(8):
        G2[:, k, :] = nn2[:, k].reshape(D2, D2)
        REP2[k, k, :] = 1.0

    f_i0 = np.zeros((C, EF), np.int64)
    f_i1 = np.zeros((C, EF), np.int64)
    ea3locT = np.zeros((C, 8, EF), np.float32)
    npc = E3 // C
    for c in range(C):
        lo = c * npc
        f_i0[c, :npc] = _pad_id_pos(ei3[0, lo:lo + npc], pos)
        f_i1[c, :npc] = _pad_id_pos(ei3[1, lo:lo + npc], pos)
        ea3locT[c, :, :npc] = g("edge_attr3")[lo:lo + npc].T

    a_nm, b_nm = _affine_bn(g("nm_g"), g("nm_be"), g("nm_m"), g("nm_v"))
    a_nm = a_nm.copy()
    a_nm[0:D2] *= 0.5
    lwWt = (g("lw_W") * a_nm[:, None]).T.astype(np.float32)   # (8,384)
    lbp = (g("lb_W")[0] + b_nm @ g("lw_W")).astype(np.float32)

    alc, blc = _affine_bn(g("lc_g"), g("lc_be"), g("lc_m"), g("lc_v"))
    W1c = g("lc_w1") * alc[None, :]
    b1c = (g("lc_w1") @ blc + g("lc_b1")).astype(np.float32)

    bih1, bhh1 = g("g1_bih"), g("g1_bhh")
    bih2, bhh2 = g("g2_bih"), g("g2_bhh")

    xs = g("x")
    in_maps = []
    for c in range(C):
        nodes = np.arange(c * NL, (c + 1) * NL)
        xp = np.empty((NL, 8), np.float32)
        xp[pos[nodes]] = xs[nodes]
        m = {
            "xT": xp.T,
            "eaT1": e1["eaT"][c],
            "gidx1": _wrap16(e1["gidx"][c]),
            "S1": e1["s"][c],
            "eaT3": e2["eaT"][c],
            "gidx3": _wrap16(e2["gidx"][c]),
            "S3": e2["s"][c],
            "gf": _wrap16(np.concatenate([f_i0[c], f_i1[c]])),
            "ea3locT": ea3locT[c],
            "Wln": Wln, "bln": bln.reshape(-1, 1),
            "leWt": g("le_W").T, "leb": g("le_b").reshape(-1, 1),
            "G1": G1.astype(bfd), "REP1": REP1.astype(bfd),
            "G2": G2.astype(bfd), "REP2": REP2.astype(bfd),
            "c1b": g("c1_b").reshape(-1, 1), "c2b": g("c2_b").reshape(-1, 1),
            "wg1": np.concatenate([
                np.concatenate([g("g1_wih").T[:, 0:D],
                                g("g1_whh").T[:, 0:D]], axis=0),
                np.concatenate([g("g1_wih").T[:, D:2 * D],
                                g("g1_whh").T[:, D:2 * D]], axis=0),
                np.concatenate([g("g1_wih").T[:, 2 * D:],
                                g("g1_whh").T[:, 2 * D:]], axis=0),
            ], axis=1).astype(bfd),
            "br1": (bih1 + bhh1)[0:D].reshape(-1, 1),
            "bz1": (bih1 + bhh1)[D:2 * D].reshape(-1, 1),
            "bin1": bih1[2 * D:].reshape(-1, 1),
            "bhn1": bhh1[2 * D:].reshape(-1, 1),
            "wih2": g("g2_wih").T.astype(bfd), "whh2": g("g2_whh").T.astype(bfd),
            "br2": (bih2 + bhh2)[0:D2].reshape(-1, 1),
            "bz2": (bih2 + bhh2)[D2:2 * D2].reshape(-1, 1),
            "bin2": bih2[2 * D2:].reshape(-1, 1),
            "bhn2": bhh2[2 * D2:].reshape(-1, 1),
            "W1cT": np.concatenate(
                [np.zeros((D, D2), np.float32), W1c.T], axis=0).astype(bfd),
            "b1c": b1c.reshape(-1, 1),
            "W2cT": g("lc_w2").T.astype(bfd), "b2c": g("lc_b2").reshape(-1, 1),
            "lwWt": lwWt, "lbp": lbp.reshape(-1, 1),
            "eye": np.eye(128, dtype=bfd),
        }
        in_maps.append({k: np.ascontiguousarray(v) for k, v in m.items()})
    static = (e1["ep"], e2["ep"], e1["runs"], e1["tcs"],
              e2["runs"], e2["tcs"])
    return static, in_maps


# ------------------------------------------------------------- kernel builder

def _build(EP1, EP3, RUNS1, TCS1, RUNS3, TCS3):
    nc = bacc.Bacc("TRN2", target_bir_lowering=False, debug=False,
                   num_devices=C)
    J1, J3 = EP1 // 128, EP3 // 128
    LZ = 9216                             # z arena (per-phase)
    LE = max(6 * EP1, 8 * EP3)            # eax arena
    LG = max(EP1, EP3)
    LM = max(J1 * 64, J3 * 128)           # msg_em arena

    def inp(name, shape, dt=F32):
        return nc.dram_tensor(name, list(shape), dt, kind="ExternalInput")

    xT = inp("xT", (8, NL))
    eaT1 = inp("eaT1", (19, EP1)); gidx1 = inp("gidx1", (128, EP1 // 16), I16)
    S1 = inp("S1", (J1, 128, NT), BF16)
    eaT3 = inp("eaT3", (8, EP3)); gidx3 = inp("gidx3", (128, EP3 // 16), I16)
    S3 = inp("S3", (J3, 128, NT), BF16)
    gf = inp("gf", (128, 2 * EF // 16), I16)
    ea3locT = inp("ea3locT", (8, EF))
    Wln = inp("Wln", (8, 64)); bln = inp("bln", (64, 1))
    leWt = inp("leWt", (19, 12)); leb = inp("leb", (12, 1))
    G1 = inp("G1", (128, 6, 64), BF16); REP1 = inp("REP1", (12, 6, 128), BF16)
    G2 = inp("G2", (128, 8, 128), BF16); REP2 = inp("REP2", (8, 8, 128), BF16)
    c1b = inp("c1b", (64, 1)); c2b = inp("c2b", (128, 1))
    wg1 = inp("wg1", (128, 192), BF16)
    br1 = inp("br1", (64, 1)); bz1 = inp("bz1", (64, 1))
    bin1 = inp("bin1", (64, 1)); bhn1 = inp("bhn1", (64, 1))
    wih2 = inp("wih2", (128, 384), BF16); whh2 = inp("whh2", (128, 384), BF16)
    br2 = inp("br2", (128, 1)); bz2 = inp("bz2", (128, 1))
    bin2 = inp("bin2", (128, 1)); bhn2 = inp("bhn2", (128, 1))
    W1cT = inp("W1cT", (128, 128), BF16); b1c = inp("b1c", (128, 1))
    W2cT = inp("W2cT", (128, 128), BF16); b2c = inp("b2c", (128, 1))
    lwWt = inp("lwWt", (8, 384)); lbp = inp("lbp", (8, 1))
    eye = inp("eye", (128, 128), BF16)
    out_f = nc.dram_tensor("out_f", [1, EF], F32, kind="ExternalOutput")

    with tile.TileContext(nc) as tc:
        with (
            tc.tile_pool(name="cst", bufs=1) as cp,
            tc.tile_pool(name="arena", bufs=1) as ar,
            tc.tile_pool(name="wk", bufs=2) as wp,
            tc.tile_pool(name="fin", bufs=1) as fp,
            tc.tile_pool(name="ps2", bufs=2, space="PSUM") as p2,
            tc.tile_pool(name="ps1", bufs=1, space="PSUM") as p1,
            tc.tile_pool(name="dram", bufs=1, space="DRAM") as dp,
        ):
            def ld(ap, shape, dt=F32, tag=None, rearr=None):
                t = cp.tile(list(shape), dt, tag=tag)
                src = ap[:] if rearr is None else ap[:].rearrange(rearr)
                nc.sync.dma_start(t[:], src)
                return t

            def lrelu_act(out, in_, bias):
                """out = rrelu(in_ + bias) via leaky-relu activation."""
                nc.scalar.activation(out, in_, AF.Prelu, bias=bias,
                                     alpha=SLOPE)

            # encode-critical consts only; the rest load after the first
            # AllGather is triggered (they hide under its latency).
            c_Wln = ld(Wln, (8, 64), tag="cWln")
            c_bln = ld(bln, (64, 1), tag="cbln")
            c_eye = ld(eye, (128, 128), BF16, "ceye")

            # arenas (slot-shared across stages)
            A_eax = ar.tile([128, LE], BF16, tag="eax")
            A_z = ar.tile([128, LZ], BF16, tag="z")
            A_g = ar.tile([128, 1, LG], BF16, tag="g")
            A_me = ar.tile([128, LM], BF16, tag="me")
            A_hf = ar.tile([128, NL], F32, tag="hf")
            A_hb = ar.tile([128, NL], BF16, tag="hb")
            A_m = ar.tile([128, NL], BF16, tag="m")
            A_mh = ar.tile([128, NL], BF16, tag="mh")  # stage1: [m;h] stack
            A_hn = ar.tile([128, NLP // 128, 128], BF16, tag="hx")

            Hf_loc = dp.tile([NLP, 128], BF16)
            H1s = [dp.tile([C * NLP, 128], BF16, addr_space="Shared",
                           name=f"H1g{i}") for i in range(2)]
            H2s = [dp.tile([C * NLP, 128], BF16, addr_space="Shared",
                           name=f"H2g{i}") for i in range(2)]
            H2f = dp.tile([C * NLP, 128], BF16, addr_space="Shared",
                          name="H2f")

            def mov_chunks(n):
                return [(j, slice(j * NT, min((j + 1) * NT, n)),
                         min((j + 1) * NT, n) - j * NT)
                        for j in range((n + NT - 1) // NT)]

            # ---------------- encode
            for j, sl, w in mov_chunks(NL):
                xt = wp.tile([8, NT], F32, tag="xt")
                nc.sync.dma_start(xt[:, 0:w], xT[:, sl])
                p = p2.tile([64, NT], F32, tag="p512")
                nc.tensor.matmul(p[:, 0:w], c_Wln[:], xt[:, 0:w],
                                 start=True, stop=True)
                lrelu_act(A_hf[0:64, sl], p[:, 0:w], c_bln[:])
                nc.vector.tensor_copy(A_mh[64:128, sl], A_hf[0:64, sl])

            RG = [list(range(C))]

            def _transpose_blocks(feat, hb, po, j0, j1):
                gw_ = 4
                for j in range(j0, j1, gw_):
                    pr = min(gw_, j1 - j)
                    pt = p1.tile([128, 512], BF16, tag="ptr")
                    for u in range(pr):
                        lo = (j + u) * 128
                        w = min(128, NL - lo)
                        nc.tensor.transpose(
                            pt[0:w, u * feat:u * feat + feat],
                            hb[po:po + feat, lo:lo + w],
                            c_eye[po:po + feat, po:po + feat])
                    pv = pt[:, 0:pr * feat].rearrange(
                        "p (u f) -> p u f", u=pr)
                    nc.vector.tensor_copy(A_hn[:, j:j + pr, 0:feat], pv)
                    if feat == 64:
                        nc.scalar.activation(A_hn[:, j:j + pr, 64:128], pv,
                                             AF.Identity)
            def export_full(feat, H, hb, po=0):
                _transpose_blocks(feat, hb, po, 0, NLP // 128)
                nc.sync.dma_start(
                    Hf_loc[:].rearrange("(j p) f -> p j f", p=128), A_hn[:])
                nc.gpsimd.collective_compute(
                    "AllGather", ALU.bypass, replica_groups=RG,
                    ins=[Hf_loc[:].opt()], outs=[H[:].opt()],
                    unique_tensors="Yes")

            def msg_pass(HH, runs, tcs, gi, EP, nk, Gc, S_d, J, feat, m_out,
                         cbias):
                GC = 512
                for off, ln, hf in runs:
                    H = HH
                    for gof in range(off, off + ln, GC):
                        gw = min(GC, off + ln - gof)
                        nc.gpsimd.dma_gather(
                            A_g[:, :, gof:gof + gw], H[:],
                            gi[:, gof // 16:(gof + gw) // 16], gw, gw, 128,
                            transpose=True)
                g2 = A_g[:].rearrange("p one e -> p (one e)")
                # Z phases: sub-slices of the gather runs, graded so the
                # first msg matmuls start right after the first gather
                # lands instead of waiting for a third of the drain
                maxph = (LZ // nk) // 128 * 128
                phases = []
                for off, ln, hf in runs:
                    o, grade = off, [256, 512, 1024]
                    gi_ = 0
                    while o < off + ln:
                        step = grade[gi_] if gi_ < len(grade) else maxph
                        gi_ += 1
                        e = min(step, off + ln - o)
                        phases.append((o, e))
                        o += e
                for base, eph in phases:
                    for kc in range(nk):
                        nc.vector.tensor_tensor(
                            A_z[:, kc * eph:(kc + 1) * eph],
                            g2[:, base:base + eph],
                            A_eax[:, kc * EP + base:kc * EP + base + eph],
                            op=ALU.mult)
                    for j, sl, w in mov_chunks(eph):
                        p = p2.tile([feat, NT], F32, tag="p512")
                        for kc in range(nk):
                            nc.tensor.matmul(
                                p[:, 0:w], Gc[:, kc, 0:feat],
                                A_z[:, kc * eph + sl.start:kc * eph + sl.stop],
                                start=(kc == 0), stop=(kc == nk - 1))
                        mc = wp.tile([feat, NT], BF16, tag="mc")
                        nc.vector.tensor_copy(mc[:, 0:w], p[:, 0:w])
                        qs, q = w // 128, 0
                        while q < qs:
                            pr = min(4, qs - q)
                            jj = (base + sl.start) // 128 + q
                            pt = p1.tile([128, 512], BF16, tag="ptr")
                            for u in range(pr):
                                nc.tensor.transpose(
                                    pt[:, u * feat:(u + 1) * feat],
                                    mc[:, (q + u) * 128:(q + u + 1) * 128],
                                    c_eye[0:feat, 0:feat])
                            nc.scalar.activation(
                                A_me[:, jj * feat:(jj + pr) * feat],
                                pt[:, 0:pr * feat], AF.Identity)
                            q += pr
                cptm = max(sum(n for _, n in sp) for sp in tcs)
                for t in range(NTC):
                    tot = sum(n for _, n in tcs[t])
                    sb = wp.tile([128, cptm, NT], BF16, tag="Sblk")
                    qi = 0
                    for cs, n in tcs[t]:
                        nc.sync.dma_start(
                            sb[:, qi:qi + n, :], S_d[cs:cs + n].rearrange(
                                "j p n -> p j n"))
                        qi += n
                    pm = p1.tile([feat, NT], F32, tag="pm")
                    qi = 0
                    for cs, n in tcs[t]:
                        for q in range(n):
                            j = cs + q
                            nc.tensor.matmul(
                                pm[:], A_me[:, j * feat:(j + 1) * feat],
                                sb[:, qi, :], start=(qi == 0),
                                stop=(qi == tot - 1))
                            qi += 1
                    hi = min(NT, NL - t * NT)
                    lrelu_act(m_out[0:feat, t * NT:t * NT + hi],
                              pm[:, 0:hi], cbias[:])

            def gru1(bR, bZ, bI, bH):
                """stage-1 GRU: m and h stacked in A_mh (128 partitions);
                R/Z gates are single K=128 matmuls against c_wg1."""
                dd = 64
                for t, sl, hi in mov_chunks(NL):
                    pR = p1.tile([dd, NT], F32, tag="pgR")
                    pZ = p1.tile([dd, NT], F32, tag="pgZ")
                    pI = p1.tile([dd, NT], F32, tag="pgI")
                    pH = p1.tile([dd, NT], F32, tag="pgH")
                    nc.tensor.matmul(pR[:, 0:hi], c_wg1[:, 0:dd],
                                     A_mh[:, sl], start=True, stop=True)
                    nc.tensor.matmul(pZ[:, 0:hi], c_wg1[:, dd:2 * dd],
                                     A_mh[:, sl], start=True, stop=True)
                    nc.tensor.matmul(pI[:, 0:hi], c_wg1[0:dd, 2 * dd:3 * dd],
                                     A_mh[0:dd, sl], start=True, stop=True)
                    nc.tensor.matmul(pH[:, 0:hi], c_wg1[dd:128, 2 * dd:3 * dd],
                                     A_mh[dd:128, sl], start=True, stop=True)
                    rs = wp.tile([dd, NT], F32, tag="grs")
                    zs = wp.tile([dd, NT], F32, tag="gzs")
                    nc.scalar.activation(rs[:, 0:hi], pR[:, 0:hi], AF.Sigmoid,
                                         bias=bR[:])
                    nc.scalar.activation(zs[:, 0:hi], pZ[:, 0:hi], AF.Sigmoid,
                                         bias=bZ[:])
                    hs = wp.tile([dd, NT], F32, tag="ghs")
                    nc.scalar.activation(hs[:, 0:hi], pH[:, 0:hi], AF.Identity,
                                         bias=bH[:])
                    t1 = wp.tile([dd, NT], F32, tag="gt1")
                    nc.vector.tensor_tensor(t1[:, 0:hi], rs[:, 0:hi],
                                            hs[:, 0:hi], op=ALU.mult)
                    nc.vector.tensor_tensor(t1[:, 0:hi], t1[:, 0:hi],
                                            pI[:, 0:hi], op=ALU.add)
                    nt_ = wp.tile([dd, NT], F32, tag="gnt")
                    nc.scalar.activation(nt_[:, 0:hi], t1[:, 0:hi], AF.Tanh,
                                         bias=bI[:])
                    hm = wp.tile([dd, NT], F32, tag="ghm")
                    nc.vector.tensor_tensor(hm[:, 0:hi], A_hf[0:dd, sl],
                                            nt_[:, 0:hi], op=ALU.subtract)
                    nc.vector.tensor_tensor(hm[:, 0:hi], hm[:, 0:hi],
                                            zs[:, 0:hi], op=ALU.mult)
                    nc.vector.tensor_tensor(A_hf[0:dd, sl], hm[:, 0:hi],
                                            nt_[:, 0:hi], op=ALU.add)
                    nc.vector.tensor_copy(A_mh[dd:128, sl], A_hf[0:dd, sl])

            def gru(dd, m_bf, wih, whh, bR, bZ, bI, bH):
                for t, sl, hi in mov_chunks(NL):
                    pR = p1.tile([dd, NT], F32, tag="pgR")
                    pZ = p1.tile([dd, NT], F32, tag="pgZ")
                    pI = p1.tile([dd, NT], F32, tag="pgI")
                    pH = p1.tile([dd, NT], F32, tag="pgH")
                    nc.tensor.matmul(pR[:, 0:hi], wih[:, 0:dd], m_bf[0:dd, sl],
                                     start=True, stop=False)
                    nc.tensor.matmul(pR[:, 0:hi], whh[:, 0:dd], A_hb[0:dd, sl],
                                     start=False, stop=True)
                    nc.tensor.matmul(pZ[:, 0:hi], wih[:, dd:2 * dd],
                                     m_bf[0:dd, sl], start=True, stop=False)
                    nc.tensor.matmul(pZ[:, 0:hi], whh[:, dd:2 * dd],
                                     A_hb[0:dd, sl], start=False, stop=True)
                    nc.tensor.matmul(pI[:, 0:hi], wih[:, 2 * dd:3 * dd],
                                     m_bf[0:dd, sl], start=True, stop=True)
                    nc.tensor.matmul(pH[:, 0:hi], whh[:, 2 * dd:3 * dd],
                                     A_hb[0:dd, sl], start=True, stop=True)
                    rs = wp.tile([dd, NT], F32, tag="grs")
                    zs = wp.tile([dd, NT], F32, tag="gzs")
                    nc.scalar.activation(rs[:, 0:hi], pR[:, 0:hi], AF.Sigmoid,
                                         bias=bR[:])
                    nc.scalar.activation(zs[:, 0:hi], pZ[:, 0:hi], AF.Sigmoid,
                                         bias=bZ[:])
                    hs = wp.tile([dd, NT], F32, tag="ghs")
                    nc.scalar.activation(hs[:, 0:hi], pH[:, 0:hi], AF.Identity,
                                         bias=bH[:])
                    t1 = wp.tile([dd, NT], F32, tag="gt1")
                    nc.vector.tensor_tensor(t1[:, 0:hi], rs[:, 0:hi],
                                            hs[:, 0:hi], op=ALU.mult)
                    nc.vector.tensor_tensor(t1[:, 0:hi], t1[:, 0:hi],
                                            pI[:, 0:hi], op=ALU.add)
                    nt_ = wp.tile([dd, NT], F32, tag="gnt")
                    nc.scalar.activation(nt_[:, 0:hi], t1[:, 0:hi], AF.Tanh,
                                         bias=bI[:])
                    hm = wp.tile([dd, NT], F32, tag="ghm")
                    nc.vector.tensor_tensor(hm[:, 0:hi], A_hf[0:dd, sl],
                                            nt_[:, 0:hi], op=ALU.subtract)
                    nc.vector.tensor_tensor(hm[:, 0:hi], hm[:, 0:hi],
                                            zs[:, 0:hi], op=ALU.mult)
                    nc.vector.tensor_tensor(A_hf[0:dd, sl], hm[:, 0:hi],
                                            nt_[:, 0:hi], op=ALU.add)
                nc.vector.tensor_copy(A_hb[0:dd, :], A_hf[0:dd, :])

            # ---------------- stage 1 (export first so AllGather overlaps
            # the const loads + edge-constant setup below)
            export_full(64, H1s[0], A_mh, po=64)

            # remaining constants (hidden under AllGather #1)
            c_leWt = ld(leWt, (19, 12), tag="cleWt")
            c_leb = ld(leb, (12, 1), tag="cleb")
            c_G1 = ld(G1, (128, 6, 64), BF16, "cG1")
            c_REP1 = ld(REP1, (12, 6, 128), BF16, "cREP1")
            c_G2 = ld(G2, (128, 8, 128), BF16, "cG2")
            c_REP2 = ld(REP2, (8, 8, 128), BF16, "cREP2")
            c_c1b = ld(c1b, (64, 1), tag="cc1b")
            c_c2b = ld(c2b, (128, 1), tag="cc2b")
            c_wg1 = ld(wg1, (128, 192), BF16, "cwg1")
            c_br1 = ld(br1, (64, 1), tag="cbr1")
            c_bz1 = ld(bz1, (64, 1), tag="cbz1")
            c_bin1 = ld(bin1, (64, 1), tag="cbin1")
            c_bhn1 = ld(bhn1, (64, 1), tag="cbhn1")
            c_wih2 = ld(wih2, (128, 384), BF16, "cwih2")
            c_whh2 = ld(whh2, (128, 384), BF16, "cwhh2")
            c_br2 = ld(br2, (128, 1), tag="cbr2")
            c_bz2 = ld(bz2, (128, 1), tag="cbz2")
            c_bin2 = ld(bin2, (128, 1), tag="cbin2")
            c_bhn2 = ld(bhn2, (128, 1), tag="cbhn2")
            c_W1cT = ld(W1cT, (128, 128), BF16, "cW1cT")
            c_b1c = ld(b1c, (128, 1), tag="cb1c")
            c_W2cT = ld(W2cT, (128, 128), BF16, "cW2cT")
            c_b2c = ld(b2c, (128, 1), tag="cb2c")
            c_lwWt = ld(lwWt, (8, 384), tag="clwWt")
            c_lbp = ld(lbp, (8, 1), tag="clbp")
            c_gi1 = ld(gidx1, (128, EP1 // 16), I16, "cgi1")
            c_gi3 = ld(gidx3, (128, EP3 // 16), I16, "cgi3")
            c_gf = ld(gf, (128, 2 * EF // 16), I16, "cgf")
            c_ones = cp.tile([128, 1], F32, tag="cones")
            nc.vector.memset(c_ones[:], 1.0)

            # stage1 edge constants: ea1 + expanded chunks
            for j, sl, w in mov_chunks(EP1):
                ea1t = wp.tile([19, NT], F32, tag="ea1t")
                nc.sync.dma_start(ea1t[:, 0:w], eaT1[:, sl])
                p = p2.tile([12, NT], F32, tag="p512")
                nc.tensor.matmul(p[:, 0:w], c_leWt[:], ea1t[:, 0:w],
                                 start=True, stop=True)
                ea1c = wp.tile([12, NT], BF16, tag="ea1c")
                lrelu_act(ea1c[:, 0:w], p[:, 0:w], c_leb[:])
                for cc in range(6):
                    pe = p2.tile([128, NT], F32, tag="p512")
                    nc.tensor.matmul(pe[:, 0:w], c_REP1[:, cc, :],
                                     ea1c[:, 0:w], start=True, stop=True)
                    dst = A_eax[:, cc * EP1 + sl.start:cc * EP1 + sl.stop]
                    if cc % 2 == 0:
                        nc.vector.tensor_copy(dst, pe[:, 0:w])
                    else:
                        nc.scalar.activation(dst, pe[:, 0:w], AF.Identity)

            for it in range(2):
                msg_pass(H1s[it], RUNS1, TCS1, c_gi1, EP1, 6, c_G1, S1, J1,
                         64, A_mh, c_c1b)
                gru1(c_br1, c_bz1, c_bin1, c_bhn1)
                if it == 0:
                    export_full(64, H1s[1], A_mh, po=64)

            # ---------------- lin_covert (h: 64 -> 128 features)
            for j, sl, w in mov_chunks(NL):
                p = p2.tile([128, NT], F32, tag="p512")
                nc.tensor.matmul(p[:, 0:w], c_W1cT[64:128, :], A_mh[64:128, sl],
                                 start=True, stop=True)
                lrelu_act(A_m[:, sl], p[:, 0:w], c_b1c[:])
            for j, sl, w in mov_chunks(NL):
                p = p2.tile([128, NT], F32, tag="p512")
                nc.tensor.matmul(p[:, 0:w], c_W2cT[:], A_m[:, sl],
                                 start=True, stop=True)
                lrelu_act(A_hf[:, sl], p[:, 0:w], c_b2c[:])
            nc.vector.tensor_copy(A_hb[:], A_hf[:])

            # ---------------- stage 2 (export first: AllGather overlaps the
            # stage2 edge-constant expansion)
            export_full(128, H2s[0], A_hb)

            # stage2 edge constants (reuse eax arena)
            for j, sl, w in mov_chunks(EP3):
                ea3f = wp.tile([8, NT], F32, tag="ea3f")
                nc.sync.dma_start(ea3f[:, 0:w], eaT3[:, sl])
                ea3b = wp.tile([8, NT], BF16, tag="ea3b")
                nc.vector.tensor_copy(ea3b[:, 0:w], ea3f[:, 0:w])
                for k in range(8):
                    p = p2.tile([128, NT], F32, tag="p512")
                    nc.tensor.matmul(p[:, 0:w], c_REP2[:, k, :], ea3b[:, 0:w],
                                     start=True, stop=True)
                    dst = A_eax[:, k * EP3 + sl.start:k * EP3 + sl.stop]
                    if k % 2 == 0:
                        nc.vector.tensor_copy(dst, p[:, 0:w])
                    else:
                        nc.scalar.activation(dst, p[:, 0:w], AF.Identity)

            # readout weight products: depend only on inputs, so compute
            # before the stage-2 iterations (hidden under AllGathers)
            ea3l = fp.tile([8, EF], F32, tag="ea3l")
            nc.sync.dma_start(ea3l[:], ea3locT[:])
            wB = fp.tile([128, 3, EF], F32, tag="wB")
            for bi in range(3):
                pw = p2.tile([128, EF], F32, tag="p512")
                nc.tensor.matmul(pw[:], c_lwWt[:, bi * 128:(bi + 1) * 128],
                                 ea3l[:], start=True, stop=True)
                nc.vector.tensor_copy(wB[:, bi, :], pw[:])

            for it in range(2):
                msg_pass(H2s[it], RUNS3, TCS3, c_gi3, EP3, 8, c_G2, S3, J3,
                         128, A_m, c_c2b)
                gru(128, A_m, c_wih2, c_whh2, c_br2, c_bz2, c_bin2, c_bhn2)
                if it == 0:
                    export_full(128, H2s[1], A_hb)
                else:
                    export_full(128, H2f, A_hb)

            # ---------------- final readout (quarter gathers: compute on
            # half A while half B is still draining)
            HQ = EF // 2
            t01 = fp.tile([128, 1, 2 * EF], BF16, tag="t01")
            qs16 = EF // 16 // 2   # idx cols per quarter
            order = [(0, 0), (2, EF), (1, HQ), (3, EF + HQ)]
            for qc, dst in order:
                nc.gpsimd.dma_gather(
                    t01[:, :, dst:dst + HQ], H2f[:],
                    c_gf[:, qc * qs16:(qc + 1) * qs16], HQ, HQ, 128,
                    transpose=True)
            fB0 = fp.tile([128, EF], F32, tag="fB0")
            fB1 = fp.tile([128, EF], F32, tag="fB1")
            fB2 = fp.tile([128, EF], F32, tag="fB2")
            fB = [fB0, fB1, fB2]
            pacc = p1.tile([1, EF], F32, tag="pm")
            for x0 in (0, HQ):
                a0 = t01[:, :, x0:x0 + HQ].rearrange("p one e -> p (one e)")
                a1 = t01[:, :, EF + x0:EF + x0 + HQ].rearrange(
                    "p one e -> p (one e)")
                nc.vector.tensor_tensor(fB[0][:, x0:x0 + HQ], a0, a1,
                                        op=ALU.add)
                nc.vector.tensor_tensor(fB[1][:, x0:x0 + HQ], a0, a1,
                                        op=ALU.mult)
                nc.vector.tensor_tensor(fB[2][:, x0:x0 + HQ], a0, a1,
                                        op=ALU.subtract)
                nc.vector.tensor_tensor(fB[2][:, x0:x0 + HQ],
                                        fB[2][:, x0:x0 + HQ],
                                        fB[2][:, x0:x0 + HQ], op=ALU.mult)
                for bi in range(3):
                    pr = fp.tile([128, EF], F32, tag="prod")
                    nc.vector.tensor_tensor(pr[:, x0:x0 + HQ],
                                            fB[bi][:, x0:x0 + HQ],
                                            wB[:, bi, x0:x0 + HQ],
                                            op=ALU.mult)
                    nc.tensor.matmul(pacc[:, x0:x0 + HQ], c_ones[:],
                                     pr[:, x0:x0 + HQ],
                                     start=(bi == 0), stop=False)
                nc.tensor.matmul(pacc[:, x0:x0 + HQ], c_lbp[:],
                                 ea3l[:, x0:x0 + HQ], start=False, stop=True)
            ot = fp.tile([1, EF], F32, tag="ot")
            nc.vector.tensor_copy(ot[:], pacc[:])
            nc.sync.dma_start(out_f[:], ot[:])

    nc.compile()
    return nc


_CACHE = {}
LAST_RESULT = None


def kernel(**inputs):
    global LAST_RESULT
    static, in_maps = _host_prep(inputs)
    if static not in _CACHE:
        _CACHE[static] = _build(*static)
    nc = _CACHE[static]
    kw = {}
    if os.environ.get("KERNEL_TRACE"):
        kw["trace"] = True
        td = os.environ.get("KERNEL_TRACE_DIR")
        if td:
            kw["tmpdir"] = td
    LAST_RESULT = run_bass_kernel_spmd(nc, in_maps, list(range(C)), **kw)
    res = LAST_RESULT.results
    return np.concatenate(
        [res[c]["out_f"][0, :E3 // C] for c in range(C)]).astype(np.float32)


# revision 76
# speedup vs baseline: 1.1301x; 1.1301x over previous
"""GNN message-passing kernel for 8 Trainium2 NeuronCores (Bass/Tile).

Sharding: each core owns 2500 nodes + all edges targeting them. Node state
is feature-major in SBUF; after each GRU update it is PE-transposed to
node-major (bf16, 128-wide rows), DMAd to DRAM and AllGathered (Shared
scratchpad) so any core can dma_gather arbitrary source rows.  Per-edge
weights are never materialized:
  msg_e = h[src_e] @ (ea_e @ nnW^T).reshape(D,D)
is computed as  Z[(k,i),e] = ea[k,e] * h[i,src_e];  msg = G^T @ Z
with G a host-prepacked rearrangement of nnW.  segment-sum over targets is
a matmul against host-built staircase blocks holding 1/deg of the target
(edges sorted by target, each 512-node tile padded to an integral number
of 128-edge chunks).
"""
import sys
sys.path.insert(0, "/opt/trn_rl_repo")
import os
import numpy as np
import ml_dtypes

import concourse.bass as bass
import concourse.bacc as bacc
import concourse.mybir as mybir
import concourse.tile as tile
from concourse.bass_utils import run_bass_kernel_spmd

F32 = mybir.dt.float32
BF16 = mybir.dt.bfloat16
I16 = mybir.dt.int16
AF = mybir.ActivationFunctionType
ALU = mybir.AluOpType

N, E, E3, D = 20000, 30000, 4000, 64
D2 = 2 * D
C = 8
NL = N // C          # nodes per core (2500)
NLP = 2560           # padded rows per core in gathered state (mult of 128)
NT = 512             # node-tile / matmul moving chunk
NTC = (NL + NT - 1) // NT
SLOPE = (1.0 / 8.0 + 1.0 / 3.0) / 2.0
EPS = 1e-5
EF = 512             # final readout edges per core (500 real)

bfd = ml_dtypes.bfloat16


# ----------------------------------------------------------------- host prep

def _wrap16(idx):
    n = len(idx)
    w = idx.reshape(n // 16, 16).T.astype(np.int16)
    return np.tile(w, (8, 1)).copy()


def _pad_id(n):
    """global node id -> padded row id in gathered state."""
    return (n // NL) * NLP + (n % NL)


def _affine_bn(g, be, m, v):
    a = g / np.sqrt(v + EPS)
    return a, be - m * a


def _balance_perm(tgt1):
    """Per-core position of each node, degree-balanced across tiles.

    Returns pos[n] in [0, NL): row of node n within its core, chosen so
    stage-1 in-edges spread evenly over the NTC 512-node tiles."""
    deg = np.bincount(tgt1, minlength=N)
    pos = np.empty(N, np.int64)
    for c in range(C):
        nodes = np.arange(c * NL, (c + 1) * NL)
        order = nodes[np.argsort(-deg[nodes], kind="stable")]
        fill = np.zeros(NTC, np.int64)
        cap = [min(NT, NL - t * NT) for t in range(NTC)]
        for i, n in enumerate(order):
            t = i % NTC
            while fill[t] >= cap[t]:
                t = (t + 1) % NTC
            pos[n] = t * NT + fill[t]
            fill[t] += 1
    return pos


HSA = 1536           # rows per rank in half-a state (tiles 0-2)
HSB = 1024           # rows per rank in half-b state (tiles 3-4, padded)


def _prep_edges(src, tgt, attr, n_attr, pos, deg):
    """Bucket edges per target tile (balanced via pos), each bucket padded
    to a 128 multiple; staircase S blocks carry 1/deg of the target."""
    owner = tgt // NL
    per_core = []
    for c in range(C):
        sel = np.where(owner == c)[0]
        tl = pos[tgt[sel]]
        sh = np.zeros(len(sel), np.int64)        # single run (no half split)
        order = np.lexsort((tl, tl // NT, sh))
        per_core.append((sel[order], tl[order], sh[order]))

    # shared static layout: per (half, tile) block counts = max over cores
    nblk = np.zeros((2, NTC), np.int64)
    for sel, tl, sh in per_core:
        for h in (0, 1):
            for t in range(NTC):
                k = int(((sh == h) & (tl // NT == t)).sum())
                nblk[h, t] = max(nblk[h, t], (k + 127) // 128)
    nblk[0] = np.maximum(nblk[0], 1)   # >=1 chunk so psum init happens
    ep = int(nblk.sum()) * 128

    # two contiguous runs (all half-a buckets, then all half-b); per tile
    # the chunk spans it owns in each half
    runs = []
    spans = [[] for _ in range(NTC)]
    off = 0
    for h in (0, 1):
        ln = int(nblk[h].sum()) * 128
        if ln:
            runs.append((off, ln, h))
        for t in range(NTC):
            n = int(nblk[h, t])
            if n:
                spans[t].append((off // 128, n))
            off += n * 128
    tcs = tuple(tuple(s) for s in spans)

    gidx = np.zeros((C, ep), np.int64)
    eaT = np.zeros((C, n_attr, ep), np.float32)
    s_blocks = np.zeros((C, ep // 128, 128, NT), bfd)

    for c, (sel, tl, sh) in enumerate(per_core):
        off = 0
        for h in (0, 1):
            for t in range(NTC):
                msk = (sh == h) & (tl // NT == t)
                idxs, tls = sel[msk], tl[msk]
                k = len(idxs)
                gidx[c, off:off + k] = _pad_id_pos(src[idxs], pos)
                eaT[c, :, off:off + k] = attr[idxs].T
                rel = tls - t * NT
                ar = np.arange(k) + off
                s_blocks[c, ar // 128, ar % 128, rel] = (
                    1.0 / deg[tgt[idxs]]).astype(bfd)
                off += int(nblk[h, t]) * 128
    return dict(ep=ep, gidx=gidx, eaT=eaT, s=s_blocks,
                runs=tuple(runs), tcs=tcs)


def _pad_id_pos(n, pos):
    """global node id -> padded row id in unsplit gathered state."""
    return (n // NL) * NLP + pos[n]


def _host_prep(inp):
    g = lambda k: np.asarray(inp[k], np.float32)
    ei = np.asarray(inp["edge_index"], np.int64)
    ei3 = np.asarray(inp["edge_index3"], np.int64)

    a, b = _affine_bn(g("nx_g"), g("nx_be"), g("nx_m"), g("nx_v"))
    Wln = (a[:, None] * g("ln_W").T).astype(np.float32)
    bln = (b @ g("ln_W").T + g("ln_b")).astype(np.float32)

    pos = _balance_perm(ei[1])
    deg1 = np.maximum(np.bincount(ei[1], minlength=N), 1).astype(np.float32)
    e1 = _prep_edges(ei[0], ei[1], g("edge_attr"), 19, pos, deg1)
    nn1 = g("nn1_W")
    G1 = np.zeros((128, 6, 64), np.float32)           # partition-first
    REP1 = np.zeros((12, 6, 128), np.float32)
    for cc in range(6):
        for half, k in enumerate((2 * cc, 2 * cc + 1)):
            G1[half * 64:(half + 1) * 64, cc, :] = nn1[:, k].reshape(64, 64)
            REP1[k, cc, half * 64:(half + 1) * 64] = 1.0

    src3 = np.concatenate([ei3[0], ei3[1]])
    tgt3 = np.concatenate([ei3[1], ei3[0]])
    attr3 = np.concatenate([g("edge_attr3"), g("edge_attr3")], axis=0)
    deg3 = np.maximum(np.bincount(tgt3, minlength=N), 1).astype(np.float32)
    e2 = _prep_edges(src3, tgt3, attr3, 8, pos, deg3)
    nn2 = g("nn2_W")
    G2 = np.zeros((128, 8, 128), np.float32)
    REP2 = np.zeros((8, 8, 128), np.float32)
    for k in range(8):
        G2[:, k, :] = nn2[:, k].reshape(D2, D2)
        REP2[k, k, :] = 1.0

    f_i0 = np.zeros((C, EF), np.int64)
    f_i1 = np.zeros((C, EF), np.int64)
    ea3locT = np.zeros((C, 8, EF), np.float32)
    npc = E3 // C
    for c in range(C):
        lo = c * npc
        f_i0[c, :npc] = _pad_id_pos(ei3[0, lo:lo + npc], pos)
        f_i1[c, :npc] = _pad_id_pos(ei3[1, lo:lo + npc], pos)
        ea3locT[c, :, :npc] = g("edge_attr3")[lo:lo + npc].T

    a_nm, b_nm = _affine_bn(g("nm_g"), g("nm_be"), g("nm_m"), g("nm_v"))
    a_nm = a_nm.copy()
    a_nm[0:D2] *= 0.5
    lwWt = (g("lw_W") * a_nm[:, None]).T.astype(np.float32)   # (8,384)
    lbp = (g("lb_W")[0] + b_nm @ g("lw_W")).astype(np.float32)

    alc, blc = _affine_bn(g("lc_g"), g("lc_be"), g("lc_m"), g("lc_v"))
    W1c = g("lc_w1") * alc[None, :]
    b1c = (g("lc_w1") @ blc + g("lc_b1")).astype(np.float32)

    bih1, bhh1 = g("g1_bih"), g("g1_bhh")
    bih2, bhh2 = g("g2_bih"), g("g2_bhh")

    xs = g("x")
    in_maps = []
    for c in range(C):
        nodes = np.arange(c * NL, (c + 1) * NL)
        xp = np.empty((NL, 8), np.float32)
        xp[pos[nodes]] = xs[nodes]
        m = {
            "xT": xp.T,
            "eaT1": e1["eaT"][c],
            "gidx1": _wrap16(e1["gidx"][c]),
            "S1": e1["s"][c],
            "eaT3": e2["eaT"][c],
            "gidx3": _wrap16(e2["gidx"][c]),
            "S3": e2["s"][c],
            "gf": _wrap16(np.concatenate([f_i0[c], f_i1[c]])),
            "ea3locT": ea3locT[c],
            "Wln": Wln, "bln": bln.reshape(-1, 1),
            "leWt": g("le_W").T, "leb": g("le_b").reshape(-1, 1),
            "G1": G1.astype(bfd), "REP1": REP1.astype(bfd),
            "G2": G2.astype(bfd), "REP2": REP2.astype(bfd),
            "c1b": g("c1_b").reshape(-1, 1), "c2b": g("c2_b").reshape(-1, 1),
            "wg1": np.concatenate([
                np.concatenate([g("g1_wih").T[:, 0:D],
                                g("g1_whh").T[:, 0:D]], axis=0),
                np.concatenate([g("g1_wih").T[:, D:2 * D],
                                g("g1_whh").T[:, D:2 * D]], axis=0),
                np.concatenate([g("g1_wih").T[:, 2 * D:],
                                g("g1_whh").T[:, 2 * D:]], axis=0),
            ], axis=1).astype(bfd),
            "br1": (bih1 + bhh1)[0:D].reshape(-1, 1),
            "bz1": (bih1 + bhh1)[D:2 * D].reshape(-1, 1),
            "bin1": bih1[2 * D:].reshape(-1, 1),
            "bhn1": bhh1[2 * D:].reshape(-1, 1),
            "wih2": g("g2_wih").T.astype(bfd), "whh2": g("g2_whh").T.astype(bfd),
            "br2": (bih2 + bhh2)[0:D2].reshape(-1, 1),
            "bz2": (bih2 + bhh2)[D2:2 * D2].reshape(-1, 1),
            "bin2": bih2[2 * D2:].reshape(-1, 1),
            "bhn2": bhh2[2 * D2:].reshape(-1, 1),
            "W1cT": np.concatenate(
                [np.zeros((D, D2), np.float32), W1c.T], axis=0).astype(bfd),
            "b1c": b1c.reshape(-1, 1),
            "W2cT": g("lc_w2").T.astype(bfd), "b2c": g("lc_b2").reshape(-1, 1),
            "lwWt": lwWt, "lbp": lbp.reshape(-1, 1),
            "eye": np.eye(128, dtype=bfd),
        }
        in_maps.append({k: np.ascontiguousarray(v) for k, v in m.items()})
    static = (e1["ep"], e2["ep"], e1["runs"], e1["tcs"],
              e2["runs"], e2["tcs"])
    return static, in_maps


# ------------------------------------------------------------- kernel builder

def _build(EP1, EP3, RUNS1, TCS1, RUNS3, TCS3):
    nc = bacc.Bacc("TRN2", target_bir_lowering=False, debug=False,
                   num_devices=C)
    J1, J3 = EP1 // 128, EP3 // 128
    LZ = 9216                             # z arena (per-phase)
    LE = max(6 * EP1, 8 * EP3)            # eax arena
    LG = max(EP1, EP3)
    LM = max(J1 * 64, J3 * 128)           # msg_em arena

    def inp(name, shape, dt=F32):
        return nc.dram_tensor(name, list(shape), dt, kind="ExternalInput")

    xT = inp("xT", (8, NL))
    eaT1 = inp("eaT1", (19, EP1)); gidx1 = inp("gidx1", (128, EP1 // 16), I16)
    S1 = inp("S1", (J1, 128, NT), BF16)
    eaT3 = inp("eaT3", (8, EP3)); gidx3 = inp("gidx3", (128, EP3 // 16), I16)
    S3 = inp("S3", (J3, 128, NT), BF16)
    gf = inp("gf", (128, 2 * EF // 16), I16)
    ea3locT = inp("ea3locT", (8, EF))
    Wln = inp("Wln", (8, 64)); bln = inp("bln", (64, 1))
    leWt = inp("leWt", (19, 12)); leb = inp("leb", (12, 1))
    G1 = inp("G1", (128, 6, 64), BF16); REP1 = inp("REP1", (12, 6, 128), BF16)
    G2 = inp("G2", (128, 8, 128), BF16); REP2 = inp("REP2", (8, 8, 128), BF16)
    c1b = inp("c1b", (64, 1)); c2b = inp("c2b", (128, 1))
    wg1 = inp("wg1", (128, 192), BF16)
    br1 = inp("br1", (64, 1)); bz1 = inp("bz1", (64, 1))
    bin1 = inp("bin1", (64, 1)); bhn1 = inp("bhn1", (64, 1))
    wih2 = inp("wih2", (128, 384), BF16); whh2 = inp("whh2", (128, 384), BF16)
    br2 = inp("br2", (128, 1)); bz2 = inp("bz2", (128, 1))
    bin2 = inp("bin2", (128, 1)); bhn2 = inp("bhn2", (128, 1))
    W1cT = inp("W1cT", (128, 128), BF16); b1c = inp("b1c", (128, 1))
    W2cT = inp("W2cT", (128, 128), BF16); b2c = inp("b2c", (128, 1))
    lwWt = inp("lwWt", (8, 384)); lbp = inp("lbp", (8, 1))
    eye = inp("eye", (128, 128), BF16)
    out_f = nc.dram_tensor("out_f", [1, EF], F32, kind="ExternalOutput")

    with tile.TileContext(nc) as tc:
        with (
            tc.tile_pool(name="cst", bufs=1) as cp,
            tc.tile_pool(name="arena", bufs=1) as ar,
            tc.tile_pool(name="wk", bufs=2) as wp,
            tc.tile_pool(name="fin", bufs=1) as fp,
            tc.tile_pool(name="ps2", bufs=2, space="PSUM") as p2,
            tc.tile_pool(name="ps1", bufs=1, space="PSUM") as p1,
            tc.tile_pool(name="dram", bufs=1, space="DRAM") as dp,
        ):
            def ld(ap, shape, dt=F32, tag=None, rearr=None):
                t = cp.tile(list(shape), dt, tag=tag)
                src = ap[:] if rearr is None else ap[:].rearrange(rearr)
                nc.sync.dma_start(t[:], src)
                return t

            def lrelu_act(out, in_, bias):
                """out = rrelu(in_ + bias) via leaky-relu activation."""
                nc.scalar.activation(out, in_, AF.Prelu, bias=bias,
                                     alpha=SLOPE)

            # encode-critical consts only; the rest load after the first
            # AllGather is triggered (they hide under its latency).
            c_Wln = ld(Wln, (8, 64), tag="cWln")
            c_bln = ld(bln, (64, 1), tag="cbln")
            c_eye = ld(eye, (128, 128), BF16, "ceye")

            # arenas (slot-shared across stages)
            A_eax = ar.tile([128, LE], BF16, tag="eax")
            A_z = ar.tile([128, LZ], BF16, tag="z")
            A_g = ar.tile([128, 1, LG], BF16, tag="g")
            A_me = ar.tile([128, LM], BF16, tag="me")
            A_hf = ar.tile([128, NL], F32, tag="hf")
            A_hb = ar.tile([128, NL], BF16, tag="hb")
            A_m = ar.tile([128, NL], BF16, tag="m")
            A_mh = ar.tile([128, NL], BF16, tag="mh")  # stage1: [m;h] stack
            A_hn = ar.tile([128, NLP // 128, 128], BF16, tag="hx")

            Hf_loc = dp.tile([NLP, 128], BF16)
            H1s = [dp.tile([C * NLP, 128], BF16, addr_space="Shared",
                           name=f"H1g{i}") for i in range(2)]
            H2s = [dp.tile([C * NLP, 128], BF16, addr_space="Shared",
                           name=f"H2g{i}") for i in range(2)]
            H2f = dp.tile([C * NLP, 128], BF16, addr_space="Shared",
                          name="H2f")

            def mov_chunks(n):
                return [(j, slice(j * NT, min((j + 1) * NT, n)),
                         min((j + 1) * NT, n) - j * NT)
                        for j in range((n + NT - 1) // NT)]

            # ---------------- encode
            for j, sl, w in mov_chunks(NL):
                xt = wp.tile([8, NT], F32, tag="xt")
                nc.sync.dma_start(xt[:, 0:w], xT[:, sl])
                p = p2.tile([64, NT], F32, tag="p512")
                nc.tensor.matmul(p[:, 0:w], c_Wln[:], xt[:, 0:w],
                                 start=True, stop=True)
                lrelu_act(A_hf[0:64, sl], p[:, 0:w], c_bln[:])
                nc.vector.tensor_copy(A_mh[64:128, sl], A_hf[0:64, sl])

            RG = [list(range(C))]

            def _transpose_blocks(feat, hb, po, j0, j1):
                gw_ = 4
                for j in range(j0, j1, gw_):
                    pr = min(gw_, j1 - j)
                    pt = p1.tile([128, 512], BF16, tag="ptr")
                    for u in range(pr):
                        lo = (j + u) * 128
                        w = min(128, NL - lo)
                        nc.tensor.transpose(
                            pt[0:w, u * feat:u * feat + feat],
                            hb[po:po + feat, lo:lo + w],
                            c_eye[po:po + feat, po:po + feat])
                    pv = pt[:, 0:pr * feat].rearrange(
                        "p (u f) -> p u f", u=pr)
                    nc.vector.tensor_copy(A_hn[:, j:j + pr, 0:feat], pv)
                    if feat == 64:
                        nc.scalar.activation(A_hn[:, j:j + pr, 64:128], pv,
                                             AF.Identity)
            def export_full(feat, H, hb, po=0):
                _transpose_blocks(feat, hb, po, 0, NLP // 128)
                nc.sync.dma_start(
                    Hf_loc[:].rearrange("(j p) f -> p j f", p=128), A_hn[:])
                nc.gpsimd.collective_compute(
                    "AllGather", ALU.bypass, replica_groups=RG,
                    ins=[Hf_loc[:].opt()], outs=[H[:].opt()])

            def msg_pass(HH, runs, tcs, gi, EP, nk, Gc, S_d, J, feat, m_out,
                         cbias):
                GC = 512
                for off, ln, hf in runs:
                    H = HH
                    for gof in range(off, off + ln, GC):
                        gw = min(GC, off + ln - gof)
                        nc.gpsimd.dma_gather(
                            A_g[:, :, gof:gof + gw], H[:],
                            gi[:, gof // 16:(gof + gw) // 16], gw, gw, 128,
                            transpose=True)
                g2 = A_g[:].rearrange("p one e -> p (one e)")
                # Z phases: sub-slices of the gather runs, graded so the
                # first msg matmuls start right after the first gather
                # lands instead of waiting for a third of the drain
                maxph = (LZ // nk) // 128 * 128
                phases = []
                for off, ln, hf in runs:
                    o, grade = off, [256, 512, 1024]
                    gi_ = 0
                    while o < off + ln:
                        step = grade[gi_] if gi_ < len(grade) else maxph
                        gi_ += 1
                        e = min(step, off + ln - o)
                        phases.append((o, e))
                        o += e
                for base, eph in phases:
                    for kc in range(nk):
                        nc.vector.tensor_tensor(
                            A_z[:, kc * eph:(kc + 1) * eph],
                            g2[:, base:base + eph],
                            A_eax[:, kc * EP + base:kc * EP + base + eph],
                            op=ALU.mult)
                    for j, sl, w in mov_chunks(eph):
                        p = p2.tile([feat, NT], F32, tag="p512")
                        for kc in range(nk):
                            nc.tensor.matmul(
                                p[:, 0:w], Gc[:, kc, 0:feat],
                                A_z[:, kc * eph + sl.start:kc * eph + sl.stop],
                                start=(kc == 0), stop=(kc == nk - 1))
                        mc = wp.tile([feat, NT], BF16, tag="mc")
                        nc.vector.tensor_copy(mc[:, 0:w], p[:, 0:w])
                        qs, q = w // 128, 0
                        while q < qs:
                            pr = min(4, qs - q)
                            jj = (base + sl.start) // 128 + q
                            pt = p1.tile([128, 512], BF16, tag="ptr")
                            for u in range(pr):
                                nc.tensor.transpose(
                                    pt[:, u * feat:(u + 1) * feat],
                                    mc[:, (q + u) * 128:(q + u + 1) * 128],
                                    c_eye[0:feat, 0:feat])
                            nc.scalar.activation(
                                A_me[:, jj * feat:(jj + pr) * feat],
                                pt[:, 0:pr * feat], AF.Identity)
                            q += pr
                cptm = max(sum(n for _, n in sp) for sp in tcs)
                for t in range(NTC):
                    tot = sum(n for _, n in tcs[t])
                    sb = wp.tile([128, cptm, NT], BF16, tag="Sblk")
                    qi = 0
                    for cs, n in tcs[t]:
                        nc.sync.dma_start(
                            sb[:, qi:qi + n, :], S_d[cs:cs + n].rearrange(
                                "j p n -> p j n"))
                        qi += n
                    pm = p1.tile([feat, NT], F32, tag="pm")
                    qi = 0
                    for cs, n in tcs[t]:
                        for q in range(n):
                            j = cs + q
                            nc.tensor.matmul(
                                pm[:], A_me[:, j * feat:(j + 1) * feat],
                                sb[:, qi, :], start=(qi == 0),
                                stop=(qi == tot - 1))
                            qi += 1
                    hi = min(NT, NL - t * NT)
                    lrelu_act(m_out[0:feat, t * NT:t * NT + hi],
                              pm[:, 0:hi], cbias[:])

            def gru1(bR, bZ, bI, bH):
                """stage-1 GRU: m and h stacked in A_mh (128 partitions);
                R/Z gates are single K=128 matmuls against c_wg1."""
                dd = 64
                for t, sl, hi in mov_chunks(NL):
                    pR = p1.tile([dd, NT], F32, tag="pgR")
                    pZ = p1.tile([dd, NT], F32, tag="pgZ")
                    pI = p1.tile([dd, NT], F32, tag="pgI")
                    pH = p1.tile([dd, NT], F32, tag="pgH")
                    nc.tensor.matmul(pR[:, 0:hi], c_wg1[:, 0:dd],
                                     A_mh[:, sl], start=True, stop=True)
                    nc.tensor.matmul(pZ[:, 0:hi], c_wg1[:, dd:2 * dd],
                                     A_mh[:, sl], start=True, stop=True)
                    nc.tensor.matmul(pI[:, 0:hi], c_wg1[0:dd, 2 * dd:3 * dd],
                                     A_mh[0:dd, sl], start=True, stop=True)
                    nc.tensor.matmul(pH[:, 0:hi], c_wg1[dd:128, 2 * dd:3 * dd],
                                     A_mh[dd:128, sl], start=True, stop=True)
                    rs = wp.tile([dd, NT], F32, tag="grs")
                    zs = wp.tile([dd, NT], F32, tag="gzs")
                    nc.scalar.activation(rs[:, 0:hi], pR[:, 0:hi], AF.Sigmoid,
                                         bias=bR[:])
                    nc.scalar.activation(zs[:, 0:hi], pZ[:, 0:hi], AF.Sigmoid,
                                         bias=bZ[:])
                    hs = wp.tile([dd, NT], F32, tag="ghs")
                    nc.scalar.activation(hs[:, 0:hi], pH[:, 0:hi], AF.Identity,
                                         bias=bH[:])
                    t1 = wp.tile([dd, NT], F32, tag="gt1")
                    nc.vector.tensor_tensor(t1[:, 0:hi], rs[:, 0:hi],
                                            hs[:, 0:hi], op=ALU.mult)
                    nc.vector.tensor_tensor(t1[:, 0:hi], t1[:, 0:hi],
                                            pI[:, 0:hi], op=ALU.add)
                    nt_ = wp.tile([dd, NT], F32, tag="gnt")
                    nc.scalar.activation(nt_[:, 0:hi], t1[:, 0:hi], AF.Tanh,
                                         bias=bI[:])
                    hm = wp.tile([dd, NT], F32, tag="ghm")
                    nc.vector.tensor_tensor(hm[:, 0:hi], A_hf[0:dd, sl],
                                            nt_[:, 0:hi], op=ALU.subtract)
                    nc.vector.tensor_tensor(hm[:, 0:hi], hm[:, 0:hi],
                                            zs[:, 0:hi], op=ALU.mult)
                    nc.vector.tensor_tensor(A_hf[0:dd, sl], hm[:, 0:hi],
                                            nt_[:, 0:hi], op=ALU.add)
                    nc.vector.tensor_copy(A_mh[dd:128, sl], A_hf[0:dd, sl])

            def gru(dd, m_bf, wih, whh, bR, bZ, bI, bH):
                for t, sl, hi in mov_chunks(NL):
                    pR = p1.tile([dd, NT], F32, tag="pgR")
                    pZ = p1.tile([dd, NT], F32, tag="pgZ")
                    pI = p1.tile([dd, NT], F32, tag="pgI")
                    pH = p1.tile([dd, NT], F32, tag="pgH")
                    nc.tensor.matmul(pR[:, 0:hi], wih[:, 0:dd], m_bf[0:dd, sl],
                                     start=True, stop=False)
                    nc.tensor.matmul(pR[:, 0:hi], whh[:, 0:dd], A_hb[0:dd, sl],
                                     start=False, stop=True)
                    nc.tensor.matmul(pZ[:, 0:hi], wih[:, dd:2 * dd],
                                     m_bf[0:dd, sl], start=True, stop=False)
                    nc.tensor.matmul(pZ[:, 0:hi], whh[:, dd:2 * dd],
                                     A_hb[0:dd, sl], start=False, stop=True)
                    nc.tensor.matmul(pI[:, 0:hi], wih[:, 2 * dd:3 * dd],
                                     m_bf[0:dd, sl], start=True, stop=True)
                    nc.tensor.matmul(pH[:, 0:hi], whh[:, 2 * dd:3 * dd],
                                     A_hb[0:dd, sl], start=True, stop=True)
                    rs = wp.tile([dd, NT], F32, tag="grs")
                    zs = wp.tile([dd, NT], F32, tag="gzs")
                    nc.scalar.activation(rs[:, 0:hi], pR[:, 0:hi], AF.Sigmoid,
                                         bias=bR[:])
                    nc.scalar.activation(zs[:, 0:hi], pZ[:, 0:hi], AF.Sigmoid,
                                         bias=bZ[:])
                    hs = wp.tile([dd, NT], F32, tag="ghs")
                    nc.scalar.activation(hs[:, 0:hi], pH[:, 0:hi], AF.Identity,
                                         bias=bH[:])
                    t1 = wp.tile([dd, NT], F32, tag="gt1")
                    nc.vector.tensor_tensor(t1[:, 0:hi], rs[:, 0:hi],
                                            hs[:, 0:hi], op=ALU.mult)
                    nc.vector.tensor_tensor(t1[:, 0:hi], t1[:, 0:hi],
                                            pI[:, 0:hi], op=ALU.add)
                    nt_ = wp.tile([dd, NT], F32, tag="gnt")
                    nc.scalar.activation(nt_[:, 0:hi], t1[:, 0:hi], AF.Tanh,
                                         bias=bI[:])
                    hm = wp.tile([dd, NT], F32, tag="ghm")
                    nc.vector.tensor_tensor(hm[:, 0:hi], A_hf[0:dd, sl],
                                            nt_[:, 0:hi], op=ALU.subtract)
                    nc.vector.tensor_tensor(hm[:, 0:hi], hm[:, 0:hi],
                                            zs[:, 0:hi], op=ALU.mult)
                    nc.vector.tensor_tensor(A_hf[0:dd, sl], hm[:, 0:hi],
                                            nt_[:, 0:hi], op=ALU.add)
                nc.vector.tensor_copy(A_hb[0:dd, :], A_hf[0:dd, :])

            # ---------------- stage 1 (export first so AllGather overlaps
            # the const loads + edge-constant setup below)
            export_full(64, H1s[0], A_mh, po=64)

            # remaining constants (hidden under AllGather #1)
            c_leWt = ld(leWt, (19, 12), tag="cleWt")
            c_leb = ld(leb, (12, 1), tag="cleb")
            c_G1 = ld(G1, (128, 6, 64), BF16, "cG1")
            c_REP1 = ld(REP1, (12, 6, 128), BF16, "cREP1")
            c_G2 = ld(G2, (128, 8, 128), BF16, "cG2")
            c_REP2 = ld(REP2, (8, 8, 128), BF16, "cREP2")
            c_c1b = ld(c1b, (64, 1), tag="cc1b")
            c_c2b = ld(c2b, (128, 1), tag="cc2b")
            c_wg1 = ld(wg1, (128, 192), BF16, "cwg1")
            c_br1 = ld(br1, (64, 1), tag="cbr1")
            c_bz1 = ld(bz1, (64, 1), tag="cbz1")
            c_bin1 = ld(bin1, (64, 1), tag="cbin1")
            c_bhn1 = ld(bhn1, (64, 1), tag="cbhn1")
            c_wih2 = ld(wih2, (128, 384), BF16, "cwih2")
            c_whh2 = ld(whh2, (128, 384), BF16, "cwhh2")
            c_br2 = ld(br2, (128, 1), tag="cbr2")
            c_bz2 = ld(bz2, (128, 1), tag="cbz2")
            c_bin2 = ld(bin2, (128, 1), tag="cbin2")
            c_bhn2 = ld(bhn2, (128, 1), tag="cbhn2")
            c_W1cT = ld(W1cT, (128, 128), BF16, "cW1cT")
            c_b1c = ld(b1c, (128, 1), tag="cb1c")
            c_W2cT = ld(W2cT, (128, 128), BF16, "cW2cT")
            c_b2c = ld(b2c, (128, 1), tag="cb2c")
            c_lwWt = ld(lwWt, (8, 384), tag="clwWt")
            c_lbp = ld(lbp, (8, 1), tag="clbp")
            c_gi1 = ld(gidx1, (128, EP1 // 16), I16, "cgi1")
            c_gi3 = ld(gidx3, (128, EP3 // 16), I16, "cgi3")
            c_gf = ld(gf, (128, 2 * EF // 16), I16, "cgf")
            c_ones = cp.tile([128, 1], F32, tag="cones")
            nc.vector.memset(c_ones[:], 1.0)

            # stage1 edge constants: ea1 + expanded chunks
            for j, sl, w in mov_chunks(EP1):
                ea1t = wp.tile([19, NT], F32, tag="ea1t")
                nc.sync.dma_start(ea1t[:, 0:w], eaT1[:, sl])
                p = p2.tile([12, NT], F32, tag="p512")
                nc.tensor.matmul(p[:, 0:w], c_leWt[:], ea1t[:, 0:w],
                                 start=True, stop=True)
                ea1c = wp.tile([12, NT], BF16, tag="ea1c")
                lrelu_act(ea1c[:, 0:w], p[:, 0:w], c_leb[:])
                for cc in range(6):
                    pe = p2.tile([128, NT], F32, tag="p512")
                    nc.tensor.matmul(pe[:, 0:w], c_REP1[:, cc, :],
                                     ea1c[:, 0:w], start=True, stop=True)
                    dst = A_eax[:, cc * EP1 + sl.start:cc * EP1 + sl.stop]
                    if cc % 2 == 0:
                        nc.vector.tensor_copy(dst, pe[:, 0:w])
                    else:
                        nc.scalar.activation(dst, pe[:, 0:w], AF.Identity)

            for it in range(2):
                msg_pass(H1s[it], RUNS1, TCS1, c_gi1, EP1, 6, c_G1, S1, J1,
                         64, A_mh, c_c1b)
                gru1(c_br1, c_bz1, c_bin1, c_bhn1)
                if it == 0:
                    export_full(64, H1s[1], A_mh, po=64)

            # ---------------- lin_covert (h: 64 -> 128 features)
            for j, sl, w in mov_chunks(NL):
                p = p2.tile([128, NT], F32, tag="p512")
                nc.tensor.matmul(p[:, 0:w], c_W1cT[64:128, :], A_mh[64:128, sl],
                                 start=True, stop=True)
                lrelu_act(A_m[:, sl], p[:, 0:w], c_b1c[:])
            for j, sl, w in mov_chunks(NL):
                p = p2.tile([128, NT], F32, tag="p512")
                nc.tensor.matmul(p[:, 0:w], c_W2cT[:], A_m[:, sl],
                                 start=True, stop=True)
                lrelu_act(A_hf[:, sl], p[:, 0:w], c_b2c[:])
            nc.vector.tensor_copy(A_hb[:], A_hf[:])

            # ---------------- stage 2 (export first: AllGather overlaps the
            # stage2 edge-constant expansion)
            export_full(128, H2s[0], A_hb)

            # stage2 edge constants (reuse eax arena)
            for j, sl, w in mov_chunks(EP3):
                ea3f = wp.tile([8, NT], F32, tag="ea3f")
                nc.sync.dma_start(ea3f[:, 0:w], eaT3[:, sl])
                ea3b = wp.tile([8, NT], BF16, tag="ea3b")
                nc.vector.tensor_copy(ea3b[:, 0:w], ea3f[:, 0:w])
                for k in range(8):
                    p = p2.tile([128, NT], F32, tag="p512")
                    nc.tensor.matmul(p[:, 0:w], c_REP2[:, k, :], ea3b[:, 0:w],
                                     start=True, stop=True)
                    dst = A_eax[:, k * EP3 + sl.start:k * EP3 + sl.stop]
                    if k % 2 == 0:
                        nc.vector.tensor_copy(dst, p[:, 0:w])
                    else:
                        nc.scalar.activation(dst, p[:, 0:w], AF.Identity)

            # readout weight products: depend only on inputs, so compute
            # before the stage-2 iterations (hidden under AllGathers)
            ea3l = fp.tile([8, EF], F32, tag="ea3l")
            nc.sync.dma_start(ea3l[:], ea3locT[:])
            wB = fp.tile([128, 3, EF], F32, tag="wB")
            for bi in range(3):
                pw = p2.tile([128, EF], F32, tag="p512")
                nc.tensor.matmul(pw[:], c_lwWt[:, bi * 128:(bi + 1) * 128],
                                 ea3l[:], start=True, stop=True)
                nc.vector.tensor_copy(wB[:, bi, :], pw[:])

            for it in range(2):
                msg_pass(H2s[it], RUNS3, TCS3, c_gi3, EP3, 8, c_G2, S3, J3,
                         128, A_m, c_c2b)
                gru(128, A_m, c_wih2, c_whh2, c_br2, c_bz2, c_bin2, c_bhn2)
                if it == 0:
                    export_full(128, H2s[1], A_hb)
                else:
                    export_full(128, H2f, A_hb)

            # ---------------- final readout (quarter gathers: compute on
            # half A while half B is still draining)
            HQ = EF // 2
            t01 = fp.tile([128, 1, 2 * EF], BF16, tag="t01")
            qs16 = EF // 16 // 2   # idx cols per quarter
            order = [(0, 0), (2, EF), (1, HQ), (3, EF + HQ)]
            for qc, dst in order:
                nc.gpsimd.dma_gather(
                    t01[:, :, dst:dst + HQ], H2f[:],
                    c_gf[:, qc * qs16:(qc + 1) * qs16], HQ, HQ, 128,
                    transpose=True)
            fB0 = fp.tile([128, EF], F32, tag="fB0")
            fB1 = fp.tile([128, EF], F32, tag="fB1")
            fB2 = fp.tile([128, EF], F32, tag="fB2")
            fB = [fB0, fB1, fB2]
            pacc = p1.tile([1, EF], F32, tag="pm")
            for x0 in (0, HQ):
                a0 = t01[:, :, x0:x0 + HQ].rearrange("p one e -> p (one e)")
                a1 = t01[:, :, EF + x0:EF + x0 + HQ].rearrange(
                    "p one e -> p (one e)")
                nc.vector.tensor_tensor(fB[0][:, x0:x0 + HQ], a0, a1,
                                        op=ALU.add)
                nc.vector.tensor_tensor(fB[1][:, x0:x0 + HQ], a0, a1,
                                        op=ALU.mult)
                nc.vector.tensor_tensor(fB[2][:, x0:x0 + HQ], a0, a1,
                                        op=ALU.subtract)
                nc.vector.tensor_tensor(fB[2][:, x0:x0 + HQ],
                                        fB[2][:, x0:x0 + HQ],
                                        fB[2][:, x0:x0 + HQ], op=ALU.mult)
                for bi in range(3):
                    pr = fp.tile([128, EF], F32, tag="prod")
                    nc.vector.tensor_tensor(pr[:, x0:x0 + HQ],
                                            fB[bi][:, x0:x0 + HQ],
                                            wB[:, bi, x0:x0 + HQ],
                                            op=ALU.mult)
                    nc.tensor.matmul(pacc[:, x0:x0 + HQ], c_ones[:],
                                     pr[:, x0:x0 + HQ],
                                     start=(bi == 0), stop=False)
                nc.tensor.matmul(pacc[:, x0:x0 + HQ], c_lbp[:],
                                 ea3l[:, x0:x0 + HQ], start=False, stop=True)
            ot = fp.tile([1, EF], F32, tag="ot")
            nc.vector.tensor_copy(ot[:], pacc[:])
            nc.sync.dma_start(out_f[:], ot[:])

    nc.compile()
    return nc


_CACHE = {}
LAST_RESULT = None


def kernel(**inputs):
    global LAST_RESULT
    static, in_maps = _host_prep(inputs)
    if static not in _CACHE:
        _CACHE[static] = _build(*static)
    nc = _CACHE[static]
    kw = {}
    if os.environ.get("KERNEL_TRACE"):
        kw["trace"] = True
        td = os.environ.get("KERNEL_TRACE_DIR")
        if td:
            kw["tmpdir"] = td
    LAST_RESULT = run_bass_kernel_spmd(nc, in_maps, list(range(C)), **kw)
    res = LAST_RESULT.results
    return np.concatenate(
        [res[c]["out_f"][0, :E3 // C] for c in range(C)]).astype(np.float32)
